# revision 9
# baseline (speedup 1.0000x reference)
"""BayesianAttention (ALiBi-style power-law prior + causal mask) on 8 trn2 cores.

Self-contained: builds a Bass/Tile kernel, shards heads across 8 NeuronCores
(2 heads per core; wq/wk/wv column-sharded, wo row-sharded), runs via
run_bass_kernel_spmd, and reduces the partial outputs on host.

Device-side layout is fully transposed (contraction dims on partitions):
  host sends x^T [c, i]; device computes q^T/k^T/v^T = W^T x^T, transposes v,
  s^T[j,i] = k^T_j . q^T_i, probs = exp(s^T + G), o^T[d,i] = v^T probs,
  out^T[e,i] = wo^T o^T.  Host returns sum_c(out^T_c)^T.
The prior bias + mask are a function of (j - i) only (Toeplitz), so they are
precomputed on host as one [128, 4096] table per head and applied with a
single DVE add per scores tile; fully-masked tiles are skipped statically.
"""

import math
import os

import ml_dtypes
import numpy as np

S = 2048          # sequence length
DIM = 2048        # model dim
H = 16            # heads
HD = 128          # head dim
N_CORES = 8
HL = H // N_CORES  # heads per core (2)
DL = HL * HD       # local projected dim (256)
IB = 512           # i-block (query block, moving free dim)
NIB = S // IB
NJT = S // 128     # key tiles of 128
GW = 4096          # G table width (needs >= S + IB + ... = 4095)
EPS = 1e-5
MASKED_THRESH = -1e8   # additive mask values below this mean "fully masked"

# matmul operand dtype: "bf16" | "f32r" | "f32"
MM_DTYPE = os.environ.get("KBA_DTYPE", "f32r")
TRACE = bool(int(os.environ.get("KBA_TRACE", "0")))

LAG = 6  # scores->o-matmul emission lag (bounds live probs tiles)

LAST_RUN_INFO = {}


# ---------------------------------------------------------------- tile patch
def _apply_tile_patch():
    """walrus CoreV3 codegen tolerates only one sync-wait on an InstDrain;
    the tile-exit drain waits on the whole global clock. Spread the waits
    across extra SP nops."""
    import concourse.tile as tile
    from concourse import mybir
    from concourse.vector_clock import ScopedClock

    if getattr(tile.TileContext, "_kba_patched", False):
        return

    def _drain_and_barrier(self, tick_clock, wait_clock):
        nc = self.nc
        drain_inst = nc.sync.drain()
        wait_clock.add_sem_waits(
            drain_inst.ins, ScopedClock({None: tick_clock.global_clock})
        )
        si = drain_inst.ins.sync_info
        waits = list(si.on_wait or [])
        if len(waits) > 1:
            si.on_wait = waits[:1]
            for i in range(1, len(waits)):
                nop = nc.sync.nop(nofuse=True)
                nop.ins.sync_info = mybir.SyncInfo(
                    on_wait=waits[i : i + 1], on_update=[]
                )
        nc.all_engine_barrier()
        assert self.sems is not None
        popped = nc._tile_sem_poison_stack.pop()
        assert popped is self._sem_poison
        nc.clear_and_free_semaphores(list(self.sems.allocated().values()))
        nc.all_engine_barrier()

    tile.TileContext._drain_and_barrier = _drain_and_barrier
    tile.TileContext._kba_patched = True

    try:
        import concourse.tile_utils as tile_utils

        tile_utils.max_sbuf_usage = 206 * 1024
    except Exception:
        pass


# ------------------------------------------------------------- host helpers
def _toeplitz_profile(m2):
    """If mask[i, j] == phi(j - i) for all i,j, return phi (length 2S-1,
    index t + S - 1), else None."""
    phi = np.empty(2 * S - 1, dtype=np.float32)
    phi[S - 1 :] = m2[0, :]
    phi[: S - 1] = m2[1:, 0][::-1]
    idx = (np.arange(S)[None, :] - np.arange(S)[:, None]) + (S - 1)
    if np.array_equal(phi[idx], m2):
        return phi
    return None


def _g_table(head, shape, scale, loc, start_pos, phi):
    """[128, GW] float32: GT[p, u] = prior(d) (+ phi(-d) if folding mask),
    where d = i - j = u - p - (S - 1)."""
    p = np.arange(128, dtype=np.int64)[:, None]
    u = np.arange(GW, dtype=np.int64)[None, :]
    d = u - p - (S - 1)          # i - j
    dist = (-d - start_pos).astype(np.float32)  # k_pos - q_pos
    sh = np.float32(shape[0, head, 0, 0])
    sc = np.float32(scale[0, head, 0, 0])
    lo = np.float32(loc[0, head, 0, 0])
    loc_t = np.float32(np.exp(lo) - np.exp(-lo))
    z = (dist - loc_t) * np.exp(sc, dtype=np.float32)
    g = -np.power(np.abs(z) + np.float32(EPS), sh, dtype=np.float32)
    if phi is not None:
        t = np.clip(-d + (S - 1), 0, 2 * S - 2)
        g = g + phi[t]
        g[(-d < -(S - 1)) | (-d > (S - 1))] = -1e9  # out of range: never read
    return np.ascontiguousarray(g.astype(np.float32))


def _kept_tiles(m2):
    """kept[ib] = list of key-tile indices jt whose [128 x IB] block is not
    fully masked. Must be identical for every core (single SPMD program)."""
    kept = []
    for ib in range(NIB):
        row = []
        for jt in range(NJT):
            blk = m2[ib * IB : (ib + 1) * IB, jt * 128 : (jt + 1) * 128]
            if blk.max() > MASKED_THRESH:
                row.append(jt)
        kept.append(row)
    return kept


# ------------------------------------------------------------ program build
_PROGRAM_CACHE = {}


def _build_program(mm_name, kept_key, fold_mask):
    key = (mm_name, kept_key, fold_mask)
    if key in _PROGRAM_CACHE:
        return _PROGRAM_CACHE[key]

    import concourse.bass as bass
    import concourse.tile as tile
    from concourse import bacc, mybir
    from concourse.masks import make_identity

    _apply_tile_patch()

    f32 = mybir.dt.float32
    if mm_name == "bf16":
        sdt = mmdt = mybir.dt.bfloat16
    else:
        sdt = f32  # storage dtype
        mmdt = mybir.dt.float32r if mm_name == "f32r" else f32

    def mm_ap(ap):
        return ap.bitcast(mmdt) if mmdt != sdt else ap

    kept = [list(row) for row in kept_key]

    nc = bacc.Bacc(
        "TRN2", target_bir_lowering=False, debug=False, num_devices=N_CORES
    )
    xT_d = nc.dram_tensor("xT", [S, S], sdt, kind="ExternalInput")
    wq_d = nc.dram_tensor("wq", [S, DL], sdt, kind="ExternalInput")
    wk_d = nc.dram_tensor("wk", [S, DL], sdt, kind="ExternalInput")
    wv_d = nc.dram_tensor("wv", [S, DL], sdt, kind="ExternalInput")
    wo_d = nc.dram_tensor("wo", [DL, S], sdt, kind="ExternalInput")
    g_d = nc.dram_tensor("g", [HL, 128, GW], f32, kind="ExternalInput")
    if not fold_mask:
        maskT_d = nc.dram_tensor("maskT", [S, S], f32, kind="ExternalInput")
    outT_d = nc.dram_tensor("outT", [S, S], f32, kind="ExternalOutput")

    Exp = mybir.ActivationFunctionType.Exp
    Copy = mybir.ActivationFunctionType.Copy

    with tile.TileContext(nc) as tc:
        import contextlib

        with contextlib.ExitStack() as ctx:
            consts = ctx.enter_context(tc.tile_pool(name="consts", bufs=1))
            xpool = ctx.enter_context(tc.tile_pool(name="xp", bufs=3))
            persist = ctx.enter_context(tc.tile_pool(name="persist", bufs=1))
            vtpool = ctx.enter_context(tc.tile_pool(name="vt", bufs=3))
            ppool = ctx.enter_context(tc.tile_pool(name="probs", bufs=LAG + 2))
            opool = ctx.enter_context(tc.tile_pool(name="oev", bufs=4))
            rpool = ctx.enter_context(tc.tile_pool(name="rp", bufs=2))
            mpool = (
                ctx.enter_context(tc.tile_pool(name="mk", bufs=3))
                if not fold_mask
                else None
            )
            mmp = ctx.enter_context(tc.tile_pool(name="mmp", bufs=6, space="PSUM"))
            tpp = ctx.enter_context(tc.tile_pool(name="tpp", bufs=1, space="PSUM"))
            sump = ctx.enter_context(tc.tile_pool(name="sump", bufs=1, space="PSUM"))

            # ---- constants / weights ----
            wq_sb = consts.tile([128, NJT, DL], sdt)
            nc.sync.dma_start(out=wq_sb[:], in_=wq_d.ap().rearrange("(ct p) d -> p ct d", p=128))
            wk_sb = consts.tile([128, NJT, DL], sdt)
            nc.sync.dma_start(out=wk_sb[:], in_=wk_d.ap().rearrange("(ct p) d -> p ct d", p=128))
            wv_sb = consts.tile([128, NJT, DL], sdt)
            nc.sync.dma_start(out=wv_sb[:], in_=wv_d.ap().rearrange("(ct p) d -> p ct d", p=128))
            wo_sb = consts.tile([128, HL, S], sdt)
            nc.sync.dma_start(out=wo_sb[:], in_=wo_d.ap().rearrange("(h p) e -> p h e", p=128))
            g_sb = consts.tile([128, HL, GW], f32)
            nc.sync.dma_start(out=g_sb[:], in_=g_d.ap().rearrange("h p u -> p h u"))
            ones_sb = consts.tile([128, 1], sdt)
            nc.vector.memset(ones_sb[:], 1.0)
            ident = consts.tile([128, 128], sdt)
            make_identity(nc, ident[:])

            qT = persist.tile([128, HL, S], sdt)   # [d, h, i]
            kT = persist.tile([128, HL, S], sdt)   # [d, h, j]
            v_sb = persist.tile([128, HL, NJT, HD], sdt)  # [j, h, jt, d]
            o_sb = persist.tile([128, HL, S], sdt)  # [d, h, i]

            # ---- phase 1: projections (q^T, k^T, v^T), v transpose ----
            for ib in range(NIB):
                isl = bass.ts(ib, IB)
                ps = {}
                for proj in range(3):
                    for dt_i in range(HL):
                        ps[(proj, dt_i)] = mmp.tile(
                            [128, IB], f32, tag="mmp", name=f"ps{proj}{dt_i}"
                        )
                for ct in range(NJT):
                    xt = xpool.tile([128, IB], sdt)
                    nc.sync.dma_start(out=xt[:], in_=xT_d[ct * 128 : (ct + 1) * 128, isl])
                    for proj, w_sb in enumerate((wq_sb, wk_sb, wv_sb)):
                        for dt_i in range(HL):
                            nc.tensor.matmul(
                                ps[(proj, dt_i)][:],
                                lhsT=mm_ap(w_sb[:, ct, dt_i * HD : (dt_i + 1) * HD]),
                                rhs=mm_ap(xt[:]),
                                start=(ct == 0),
                                stop=(ct == NJT - 1),
                            )
                for dt_i in range(HL):
                    nc.scalar.activation(qT[:, dt_i, isl], ps[(0, dt_i)][:], Copy)
                    nc.vector.tensor_copy(kT[:, dt_i, isl], ps[(1, dt_i)][:])
                    vt = vtpool.tile([128, IB], sdt)
                    nc.vector.tensor_copy(vt[:], ps[(2, dt_i)][:])
                    # transpose v^T [d, j] -> v [j, d] in 128-blocks
                    for s4 in range(IB // 128):
                        jt = (ib * IB) // 128 + s4
                        tp = tpp.tile([128, 128], sdt, tag="tpp")
                        nc.tensor.transpose(
                            tp[:], vt[:, s4 * 128 : (s4 + 1) * 128], ident[:]
                        )
                        nc.vector.tensor_copy(v_sb[:, dt_i, jt, :], tp[:])

            # ---- phase 2: attention per local head ----
            for h in range(HL):
                for ib in range(NIB):
                    isl = bass.ts(ib, IB)
                    jts = kept[ib]
                    oacc = tpp.tile([128, IB], f32, tag="tpp")
                    sacc = sump.tile([1, IB], f32, tag="sump")
                    probs = {}

                    def emit_pv(idx):
                        jt = jts[idx]
                        nc.tensor.matmul(
                            oacc[:],
                            lhsT=mm_ap(v_sb[:, h, jt, :]),
                            rhs=mm_ap(probs[idx][:]),
                            start=(idx == 0),
                            stop=(idx == len(jts) - 1),
                        )
                        nc.tensor.matmul(
                            sacc[:],
                            lhsT=mm_ap(ones_sb[:]),
                            rhs=mm_ap(probs[idx][:]),
                            start=(idx == 0),
                            stop=(idx == len(jts) - 1),
                        )

                    for idx, jt in enumerate(jts):
                        sc = mmp.tile([128, IB], f32, tag="mmp")
                        nc.tensor.matmul(
                            sc[:],
                            lhsT=mm_ap(kT[:, h, jt * 128 : (jt + 1) * 128]),
                            rhs=mm_ap(qT[:, h, isl]),
                            start=True,
                            stop=True,
                        )
                        base = ib * IB - jt * 128 + (S - 1)
                        nc.vector.tensor_add(
                            sc[:], sc[:], g_sb[:, h, base : base + IB]
                        )
                        if not fold_mask:
                            mt = mpool.tile([128, IB], f32)
                            nc.sync.dma_start(
                                out=mt[:],
                                in_=maskT_d[jt * 128 : (jt + 1) * 128, isl],
                            )
                            nc.vector.tensor_add(sc[:], sc[:], mt[:])
                        pb = ppool.tile([128, IB], sdt)
                        nc.scalar.activation(pb[:], sc[:], Exp)
                        probs[idx] = pb
                        if idx - LAG >= 0:
                            emit_pv(idx - LAG)
                    for idx in range(max(0, len(jts) - LAG), len(jts)):
                        emit_pv(idx)

                    rcp = rpool.tile([1, IB], f32, tag="rcp")
                    nc.vector.reciprocal(rcp[:], sacc[:])
                    rbc = rpool.tile([128, IB], f32, tag="rbc")
                    nc.gpsimd.partition_broadcast(rbc[:], rcp[:])
                    nc.vector.tensor_mul(o_sb[:, h, isl], oacc[:], rbc[:])

            # ---- phase 3: out^T = wo^T @ o^T (partial; host sums cores) ----
            for et in range(NJT):
                for ib in range(NIB):
                    isl = bass.ts(ib, IB)
                    po = mmp.tile([128, IB], f32, tag="mmp")
                    for h in range(HL):
                        nc.tensor.matmul(
                            po[:],
                            lhsT=mm_ap(wo_sb[:, h, et * 128 : (et + 1) * 128]),
                            rhs=mm_ap(o_sb[:, h, isl]),
                            start=(h == 0),
                            stop=(h == HL - 1),
                        )
                    ot = opool.tile([128, IB], f32)
                    if (et + ib) % 2 == 0:
                        nc.scalar.activation(ot[:], po[:], Copy)
                    else:
                        nc.vector.tensor_copy(ot[:], po[:])
                    nc.gpsimd.dma_start(
                        out=outT_d[et * 128 : (et + 1) * 128, isl], in_=ot[:]
                    )

    nc.compile()
    _PROGRAM_CACHE[key] = nc
    return nc


# ------------------------------------------------------------------- kernel
def prepare(x, mask, wq, wk, wv, wo, shape, scale, loc, start_pos):
    """Host prep: build/cache program and per-core input maps."""
    mm_name = MM_DTYPE
    np_store = ml_dtypes.bfloat16 if mm_name == "bf16" else np.float32

    x32 = np.asarray(x, np.float32).reshape(S, DIM)
    m2 = np.asarray(mask, np.float32).reshape(S, S)
    wq32 = np.asarray(wq, np.float32)
    wk32 = np.asarray(wk, np.float32)
    wv32 = np.asarray(wv, np.float32)
    wo32 = np.asarray(wo, np.float32)
    shape = np.asarray(shape, np.float32)
    scale = np.asarray(scale, np.float32)
    loc = np.asarray(loc, np.float32)
    sp = int(start_pos)

    phi = _toeplitz_profile(m2)
    fold_mask = phi is not None
    kept = _kept_tiles(m2)
    kept_key = tuple(tuple(row) for row in kept)

    nc = _build_program(mm_name, kept_key, fold_mask)

    xT = np.ascontiguousarray(x32.T).astype(np_store)
    inv_s = np.float32(1.0 / math.sqrt(HD))
    maskT = None if fold_mask else np.ascontiguousarray(m2.T)

    in_maps = []
    for c in range(N_CORES):
        sl = slice(c * DL, (c + 1) * DL)
        im = {
            "xT": xT,
            "wq": np.ascontiguousarray(wq32[:, sl] * inv_s).astype(np_store),
            "wk": np.ascontiguousarray(wk32[:, sl]).astype(np_store),
            "wv": np.ascontiguousarray(wv32[:, sl]).astype(np_store),
            "wo": np.ascontiguousarray(wo32[sl, :]).astype(np_store),
            "g": np.stack(
                [
                    _g_table(c * HL + h, shape, scale, loc, sp, phi)
                    for h in range(HL)
                ]
            ),
        }
        if not fold_mask:
            im["maskT"] = maskT
        in_maps.append(im)
    return nc, in_maps


def _reduce(results):
    acc = results[0]["outT"].astype(np.float32)
    for c in range(1, N_CORES):
        acc = acc + results[c]["outT"]
    return np.ascontiguousarray(acc.T)[None].astype(np.float32)


def kernel(x, mask, wq, wk, wv, wo, shape, scale, loc, start_pos):
    from concourse.bass_utils import run_bass_kernel_spmd

    nc, in_maps = prepare(x, mask, wq, wk, wv, wo, shape, scale, loc, start_pos)

    if os.environ.get("KBA_SIM", "0") == "1":
        from concourse import bass_interp

        n_sim = int(os.environ.get("KBA_SIM_CORES", str(N_CORES)))
        sim = bass_interp.MultiCoreSim(nc, n_sim)
        for c in range(n_sim):
            for k, v in in_maps[c].items():
                sim.cores[c].tensor(k)[:] = v
        sim.simulate()
        results = [
            {"outT": np.array(sim.cores[c].tensor("outT"), np.float32)}
            for c in range(n_sim)
        ] + [
            {"outT": np.zeros((S, S), np.float32)} for _ in range(N_CORES - n_sim)
        ]
        LAST_RUN_INFO["exec_time_ns"] = None
    else:
        res = run_bass_kernel_spmd(
            nc, in_maps, core_ids=list(range(N_CORES)), trace=False
        )
        LAST_RUN_INFO["exec_time_ns"] = res.exec_time_ns
        results = res.results

    LAST_RUN_INFO["results"] = results
    return _reduce(results)
